# revision 1
# baseline (speedup 1.0000x reference)
"""Multi-head attention (B=4, N=2048, D=1024, H=16) on 8 Trainium2 cores — v2.

Sharding: core = (batch b, head-group hg) -> 4 batches x 2 groups of 8 heads.

Cost-model-driven redesign vs v1 (422us -> ~320us):
  - All matmuls bf16 (1.0 cycles/row at any free size in the cost model;
    halves DMA + SBUF vs fp32r).
  - PV restructured: O[qtok,feat] = (P^T)^T @ V per 128-qtok chunk with the
    P^T block as the (cost-free) stationary operand -> 8x64 + 8x1 rows per
    (j, unit) instead of the v1 O^T formulation's 2x1024 rows. Denominators
    via 1-row ones-matmuls into a persistent PSUM bank; single-op
    per-partition normalize (broadcast reciprocal) on DVE; O transposed
    back to [feat,qtok] with cheap PE transposes for the projection.
  - PSUM zero regions are whole 2KB banks (start=True zeroes the bank), so
    multi-group accumulator banks (O, denominators, transpose quarters) are
    zeroed once by a dummy matmul of the all-zeros const and every real
    matmul accumulates (start=False, skip_group_check); the dummy's full-
    bank write also supplies the RAW ordering dependency.
  - V bias folded into the host epilogue: y += b_qkv[2D:] @ W_proj
    (exact: softmax weights sum to 1).
  - The exp stream (ACT engine, 256 exps of [128,1024] ~= 267us) is the
    pacing engine. A minimal prefix (K m0 n0, Q m0 i0, V h0 g01) starts it
    ~15us in; all remaining QKV/proj work is emitted as deadline-sorted PE
    fillers inside the attention steps, credit-paced to the ACT rate, with
    deadline-forced pops (tile deps follow emission order — a consumer
    emitted before its producer gets no dependency).
  - Unit order interleaves i-blocks per head-pair so the i1 units (which
    need no new K/V fillers) relieve the i0 filler pressure; PV trails the
    exp stream by 3 steps and transposes trail the normalize by 3 more so
    the previous unit's normalize chain never head-of-line blocks scores.
  - 12 warmup matmuls on a const tile at t~0 absorb the PE p-state ramp
    (LOW/MID cycles until 3us of continuous busy) under the initial DMA
    wait; idle gaps stay < 3us so the ramp never resets.
  - Host pre-swizzles weights into per-partition-contiguous layouts so
    every DMA moves >=1KB runs (no <512B descriptor penalty); y output in
    bf16 halves the tail writeback.

PSUM budget (8 banks): s 2x[128,1024]=4, o [128,512]=1, dn [128,512]=1
(denominators in cols 0:128 = 16 units x 8 qtok-chunks), y 2x[128,512]=2
(K/Q/V/proj fillers + transposes, rotating).
"""

import sys

if "/opt/trn_rl_repo" not in sys.path:
    sys.path.insert(0, "/opt/trn_rl_repo")

from collections import deque
from contextlib import ExitStack

import numpy as np

B, N, D, H = 4, 2048, 1024, 16
HG = 2                 # head groups (tensor parallel)
NCORES = B * HG        # 8
DH = D // HG           # 512 features per group = 8 heads * 64
HH = H // HG           # 8 heads per core
P = 128
KC = D // P            # 8 contraction chunks over d_model
CP = HH // 2           # 4 head pairs per core
TJ = N // P            # 16 key 128-chunks
IB = 1024              # i-block (exp free-dim)
NI = N // IB           # 2
SCALE = (D // H) ** -0.5

_cached = {}


def _build():
    import concourse.mybir as mybir
    import concourse.tile as tile
    from concourse import bacc, masks

    f32 = mybir.dt.float32
    bf16 = mybir.dt.bfloat16
    AF = mybir.ActivationFunctionType

    nc = bacc.Bacc("TRN2", target_bir_lowering=False, debug=False,
                   enable_asserts=False)

    # weights arrive pre-swizzled from the host into per-partition-contiguous
    # layouts so every DMA moves >=1KB-contiguous runs (no <512B descriptor
    # penalty): wq/wk [CP, P, KC, 128] (m-chunk major), wv [HH, P, KC, 64],
    # wp [P, CP, D].
    xt = nc.dram_tensor("xt", (D, N), bf16, kind="ExternalInput").ap()
    wqh = nc.dram_tensor("wqh", (CP, P, KC, P), bf16, kind="ExternalInput").ap()
    wkh = nc.dram_tensor("wkh", (CP, P, KC, P), bf16, kind="ExternalInput").ap()
    wvh = nc.dram_tensor("wvh", (HH, P, KC, 64), bf16, kind="ExternalInput").ap()
    wph = nc.dram_tensor("wph", (P, CP, D), bf16, kind="ExternalInput").ap()
    bqk = nc.dram_tensor("bqk", (1, 2 * DH), f32, kind="ExternalInput").ap()
    y = nc.dram_tensor("y", (N, D), bf16, kind="ExternalOutput").ap()

    xt_r = xt.rearrange("(ko p) t -> p ko t", p=P)

    with tile.TileContext(nc) as tc, ExitStack() as ctx:
        const = ctx.enter_context(tc.tile_pool(name="const", bufs=1))
        persist = ctx.enter_context(tc.tile_pool(name="persist", bufs=1))
        ppool = ctx.enter_context(tc.tile_pool(name="pp", bufs=8))
        ospool = ctx.enter_context(tc.tile_pool(name="osb", bufs=2))
        otpool = ctx.enter_context(tc.tile_pool(name="ot", bufs=2))
        dpool = ctx.enter_context(tc.tile_pool(name="dv", bufs=2))
        ypool = ctx.enter_context(tc.tile_pool(name="yb", bufs=5))
        psp = ctx.enter_context(tc.tile_pool(name="psp", bufs=1, space="PSUM"))

        # ---- consts ----
        wconst = const.tile([P, 512], bf16)
        nc.vector.memset(wconst[:], 0.0)
        ones_bf = const.tile([P, 16], bf16)
        nc.vector.memset(ones_bf[:], 1.0)

        # ---- persistent SBUF ----
        xt_sb = persist.tile([P, KC, N], bf16)
        wk_sb = persist.tile([P, CP, KC, P], bf16)   # [p, m, k, 128]
        wq_sb = persist.tile([P, CP, KC, P], bf16)
        wv_sb = persist.tile([P, HH, KC, 64], bf16)  # [p, h, k, 64]
        wp_sb = persist.tile([P, CP, D], bf16)
        kt = persist.tile([P, CP, N], bf16)        # K^T [feat128(pair), c, keytok]
        qt = persist.tile([P, CP, N], bf16)        # Q^T [feat128(pair), c, qtok]
        vsb = persist.tile([P, TJ, HH, 64], bf16)  # V [keytok128, j, h, feat]
        bqk_sb = const.tile([P, 1, 2 * CP], f32)

        # ---- DMAs (gpsimd queue: cheap issue), prefix-critical first ----
        nc.sync.dma_start(wk_sb[:, 0], wkh[0])
        nc.sync.dma_start(xt_sb[:, :, 0:512], xt_r[:, :, 0:512])
        nc.sync.dma_start(wq_sb[:, 0], wqh[0])
        nc.sync.dma_start(xt_sb[:, :, 512:1024], xt_r[:, :, 512:1024])
        nc.sync.dma_start(wv_sb[:, 0], wvh[0])
        nc.sync.dma_start(bqk_sb[:], bqk.rearrange("a (mo p) -> p a mo", p=P))
        nc.sync.dma_start(xt_sb[:, :, 1024:1536], xt_r[:, :, 1024:1536])
        nc.sync.dma_start(xt_sb[:, :, 1536:2048], xt_r[:, :, 1536:2048])
        nc.sync.dma_start(wv_sb[:, 1:HH], wvh[1:HH].rearrange("h p k f -> p h k f"))
        nc.sync.dma_start(wk_sb[:, 1:CP], wkh[1:CP].rearrange("m p k f -> p m k f"))
        nc.sync.dma_start(wq_sb[:, 1:CP], wqh[1:CP].rearrange("m p k f -> p m k f"))
        nc.sync.dma_start(wp_sb[:], wph)

        # identity for PE transposes (gpsimd, after the DMA issues)
        ident_f32 = const.tile([P, P], f32)
        masks.make_identity(nc, ident_f32[:])

        # preload the exp table while ACT is idle
        dummy = const.tile([1, 16], f32)
        nc.scalar.activation(dummy[:], ones_bf[0:1, :], AF.Exp)

        # persistent PSUM: denominators. Zeroed once (see emit_pv on the
        # zero-region constraint); every denom matmul accumulates.
        dn = psp.tile([P, 512], f32, tag="d", bufs=1, name="dn")
        # zero via a PE matmul of the all-zeros const (start=True also zeroes
        # the whole 2KB region); gives every denom matmul a RAW ordering dep
        nc.tensor.matmul(dn[:, 0:NI * HH * 8], wconst[:, 0:P],
                         wconst[:, 0:NI * HH * 8], start=True, stop=False,
                         skip_group_check=True)

        # warmup: absorb the PE p-state ramp before real work dispatches.
        # Rotating through the "s" slots chains them (WAW) ahead of the real
        # work that reuses those slots, so the scheduler cannot defer them.
        for _ in range(12):
            wm = psp.tile([P, IB], f32, tag="s", bufs=2, name="wm")
            nc.tensor.matmul(wm[:, 0:512], wconst[:, 0:P], wconst[:],
                             start=True, stop=True, skip_group_check=True)

        # ================= emission helpers =================
        def _bufs(tag):
            return 2 if tag in ("s", "y") else 1

        # K/Q/V filler PSUM: manual rotating subtile slots in the spare
        # columns of the dn bank (cols 128:512). Tile subtile-dep tracking
        # supplies the WARs; distinct column ranges keep the accumulation
        # groups independent of the denominators in cols 0:128.
        kq_slot = [0]
        v_slot = [0]

        def _kq_psum(tag):
            return psp.tile([P, 512], f32, tag=tag, bufs=_bufs(tag),
                            name="ptkq")[:, 0:256]

        def _v_psum(tag):
            return psp.tile([P, 512], f32, tag=tag, bufs=_bufs(tag),
                            name="ptv")[:, 0:64]

        def emit_k_chunk(m, n, tag, half=None):
            """kt[:, m, n*512+...] — 8 matmuls + bias-add copy."""
            t0, tw = (0, 512) if half is None else (half * 256, 256)
            c0 = n * 512 + t0
            if tw == 512:
                pt = psp.tile([P, 512], f32, tag=tag, bufs=_bufs(tag),
                              name="ptk")
            else:
                pt = _kq_psum(tag)
            for k in range(KC):
                nc.tensor.matmul(pt[:, 0:tw], wk_sb[:, m, k, :],
                                 xt_sb[:, k, c0:c0 + tw],
                                 start=(k == 0), stop=(k == KC - 1))
            nc.vector.tensor_scalar_add(
                kt[:, m, c0:c0 + tw], pt[:, 0:tw],
                bqk_sb[:, 0, CP + m:CP + m + 1])

        def emit_q_chunk(m, n, tag, half=None):
            t0, tw = (0, 512) if half is None else (half * 256, 256)
            c0 = n * 512 + t0
            if tw == 512:
                pt = psp.tile([P, 512], f32, tag=tag, bufs=_bufs(tag),
                              name="ptq")
            else:
                pt = _kq_psum(tag)
            for k in range(KC):
                nc.tensor.matmul(pt[:, 0:tw], wq_sb[:, m, k, :],
                                 xt_sb[:, k, c0:c0 + tw],
                                 start=(k == 0), stop=(k == KC - 1))
            nc.vector.tensor_scalar_add(
                qt[:, m, c0:c0 + tw], pt[:, 0:tw],
                bqk_sb[:, 0, m:m + 1])

        def emit_v_slice(h, g, tag):
            """vsb[:, g, h, :] — V features of head h for keytok chunk g."""
            n, tt = divmod(g, 4)
            pv = _v_psum(tag)
            for k in range(KC):
                nc.tensor.matmul(
                    pv[:],
                    xt_sb[:, k, n * 512 + tt * P:n * 512 + (tt + 1) * P],
                    wv_sb[:, h, k, :],
                    start=(k == 0), stop=(k == KC - 1))
            nc.vector.tensor_copy(vsb[:, g, h, :], pv[:])

        ot_tiles = {}

        def emit_proj(i, t, o, tag):
            ot_i = ot_tiles[i]
            yp = psp.tile([P, 512], f32, tag=tag, bufs=_bufs(tag), name="yp")
            for cc in range(CP):
                nc.tensor.matmul(yp[:], ot_i[:, cc, t * P:(t + 1) * P],
                                 wp_sb[:, cc, o * 512:(o + 1) * 512],
                                 start=(cc == 0), stop=(cc == CP - 1))
            ysb = ypool.tile([P, 512], bf16, tag="ysb", name="ysb")
            nc.vector.tensor_copy(ysb[:], yp[:])
            r0 = i * IB + t * P
            nc.sync.dma_start(y[r0:r0 + P, o * 512:(o + 1) * 512], ysb[:])

        # filler queue: (deadline_step, rows, fn(tag)) in deadline order.
        # Credit paces early pops; the deadline FORCES emission before the
        # consuming instruction is emitted (tile deps follow emission order —
        # a consumer emitted before its producer gets no dependency at all).
        fillers = deque()

        def F(rows, fn, deadline=10**9):
            fillers.append((deadline, rows, fn))

        def pump(credit, step=-1):
            while fillers and (fillers[0][0] <= step
                               or fillers[0][1] <= credit):
                _, rows, fn = fillers.popleft()
                fn("y")
                credit -= rows
            return credit

        # ================= prefix =================
        emit_k_chunk(0, 0, "s")
        emit_q_chunk(0, 0, "s")
        emit_q_chunk(0, 1, "s")
        emit_v_slice(0, 0, "s")
        emit_v_slice(0, 1, "s")

        # Pair-interleaved unit order: the i1 units of a pair consume no new
        # K/V fillers, so spreading them between i0 pairs removes the
        # i0-phase PE deficit.
        NU = NI * HH                      # 16 units
        # i0 pairs 0/1 interleave with their i1 twins; i0 pairs 2/3 run
        # back-to-back so i0 completes at position 11 and proj(0) fully
        # drains inside positions 12-15; tail is proj(1) only.
        seq = [(0, 0), (0, 1), (1, 0), (1, 1),
               (0, 2), (0, 3), (1, 2), (1, 3),
               (0, 4), (0, 5), (0, 6), (0, 7),
               (1, 4), (1, 5), (1, 6), (1, 7)]
        pos_of = {u: p for p, u in enumerate(seq)}

        # ---- filler queue: deadline-sorted fine-grained items ----
        # global steps: unit u spans steps 16u..16u+15 (u = 8i + h)
        events = []   # (deadline_step, rows, fn)
        for h in range(HH):
            for g in range(TJ):
                if h == 0 and g < 2:
                    continue   # prefix
                events.append((16 * pos_of[(0, h)] + g, 512,
                               lambda tag, h=h, g=g: emit_v_slice(h, g, tag)))
        for c in range(CP):
            p0 = 16 * pos_of[(0, 2 * c)]
            p1 = 16 * pos_of[(1, 2 * c)]
            for n in range(4):
                for half in range(2):
                    if c == 0 and n == 0:
                        continue   # prefix
                    events.append((max(0, p0 + 4 * n - 2), 2048,
                                   lambda tag, c=c, n=n, hf=half:
                                   emit_k_chunk(c, n, tag, hf)))
            for n in range(4):
                for half in range(2):
                    if c == 0 and n < 2:
                        continue   # prefix
                    dl = p0 - 2 if n < 2 else p1 - 2
                    events.append((max(0, dl), 2048,
                                   lambda tag, c=c, n=n, hf=half:
                                   emit_q_chunk(c, n, tag, hf)))
        events.sort(key=lambda e: e[0])
        for dl, rows, fn in events:
            F(rows, fn, deadline=dl)

        # ================= attention units =================
        p_of = {}      # (u, j) -> p tile
        o_ps_of = {}   # u -> O psum accumulator
        osb_of = {}    # (i, c) -> normalized-O sbuf tile

        def emit_scores_exp(u, j):
            i, h = seq[u]
            c, hp = divmod(h, 2)
            r0, r1 = hp * 64, hp * 64 + 64
            s = psp.tile([P, IB], f32, tag="s", bufs=2, name="s")
            for iq in range(2):
                nc.tensor.matmul(
                    s[:, iq * 512:(iq + 1) * 512],
                    kt[r0:r1, c, j * P:(j + 1) * P],
                    qt[r0:r1, c, i * IB + iq * 512:i * IB + (iq + 1) * 512],
                    start=True, stop=True)
            p = ppool.tile([P, IB], bf16, tag="p", name="p")
            nc.scalar.activation(p[:], s[:], AF.Exp, scale=SCALE)
            p_of[(u, j)] = p

        def emit_pv(u, j):
            i, h = seq[u]
            if j == 0:
                # PSUM "zero regions" are whole 2KB banks: start=True zeroes
                # the entire bank, so per-column accumulation groups are
                # impossible. Zero the bank once via DVE memset and run every
                # matmul with start=False (accumulate); the memset's RAW dep
                # also pins the ordering.
                o_ps_of[u] = psp.tile([P, 512], f32, tag="o", bufs=1,
                                      name="ops")
                nc.tensor.matmul(o_ps_of[u][:], wconst[:, 0:P], wconst[:],
                                 start=True, stop=False,
                                 skip_group_check=True)
            o_ps = o_ps_of[u]
            p = p_of.pop((u, j))
            for t in range(8):
                nc.tensor.matmul(o_ps[:, t * 64:(t + 1) * 64],
                                 p[:, t * P:(t + 1) * P], vsb[:, j, h, :],
                                 start=False, stop=False,
                                 skip_group_check=True)
            for t in range(8):
                nc.tensor.matmul(dn[:, u * 8 + t:u * 8 + t + 1],
                                 p[:, t * P:(t + 1) * P], ones_bf[:, 0:1],
                                 start=False, stop=False,
                                 skip_group_check=True)

        def emit_norm(u):
            """Normalize O of unit u into osb (per-partition 1/denom)."""
            i, h = seq[u]
            c, hp = divmod(h, 2)
            o_ps = o_ps_of.pop(u)
            rcp = dpool.tile([P, 8], f32, tag="rcp", name="rcp")
            nc.vector.reciprocal(rcp[:], dn[:, u * 8:u * 8 + 8])
            if hp == 0:
                osb_of[(i, c)] = ospool.tile([P, 8, 2, 64], f32, tag="osb",
                                             name="osb")
            osb = osb_of[(i, c)]
            nc.vector.tensor_mul(
                osb[:, :, hp, :],
                o_ps[:].rearrange("p (t f) -> p t f", f=64),
                rcp[:, :, None].broadcast_to([P, 8, 64]))

        def emit_transposes(i, c):
            """O pair-block [qtok, 128feat] -> ot [128feat, qtok] via PE."""
            osb = osb_of.pop((i, c))
            for g in range(2):
                yslot = psp.tile([P, 512], f32, tag="y", bufs=2, name="tp")
                nc.tensor.matmul(yslot[:], wconst[:, 0:P], wconst[:],
                                 start=True, stop=False,
                                 skip_group_check=True)
                for tt in range(4):
                    t = g * 4 + tt
                    nc.tensor.matmul(
                        yslot[:, tt * P:(tt + 1) * P],
                        osb[:, t, :, :].rearrange("p a b -> p (a b)"),
                        ident_f32[:], is_transpose=True,
                        start=False, stop=False, skip_group_check=True)
                nc.vector.tensor_copy(
                    ot_tiles[i][:, c, g * 512:(g + 1) * 512],
                    yslot[:, 0:512])

        # PV trails the scores/exp stream by 3 steps (PV j0 by 4, paired with
        # j1) and transposes trail the norm by 3 more, so nothing that waits
        # on the previous unit's normalize sits near the boundary in the PE
        # instruction order — the norm chain latency is absorbed by buffered
        # exps instead of head-of-line blocking the scores.
        CREDIT_PER_STEP = 890
        CREDIT_CAP = 4200
        PVLAG = 5
        credit = -3000    # delay the first credit pops past the prefix chain
        pending = {}
        tp_done = {0: 0, 1: 0}
        for g in range(NU * TJ + PVLAG + 4):
            credit = pump(credit, g)   # deadline-forced pops
            if g < NU * TJ:
                u, j = divmod(g, TJ)
                if j == 0:
                    i, h = seq[u]
                    if h == 0 and i not in ot_tiles:
                        ot_tiles[i] = otpool.tile([P, CP, IB], bf16,
                                                  tag="ot", name="ot")
                emit_scores_exp(u, j)
            gp = g - PVLAG
            if 0 <= gp < NU * TJ:
                up, jp = divmod(gp, TJ)
                if jp == 0:
                    pass                      # deferred: paired with j1
                elif jp == 1:
                    emit_pv(up, 0)
                    emit_pv(up, 1)
                else:
                    emit_pv(up, jp)
                if jp == TJ - 1:
                    iup, hup = seq[up]
                    emit_norm(up)
                    if hup % 2 == 1:
                        pending.setdefault(g + 6, []).append(
                            ("tp", iup, hup // 2))
            for kind, a1, a2 in pending.pop(g, []):
                emit_transposes(a1, a2)
                tp_done[a1] += 1
                if tp_done[a1] == CP:
                    # all of i-block a1's ot written: queue its projection
                    for t in range(8):
                        for o in range(2):
                            F(2048, lambda tag, ii=a1, tt=t, oo=o:
                              emit_proj(ii, tt, oo, tag))
            credit = min(credit + CREDIT_PER_STEP, CREDIT_CAP)
            credit = pump(credit, g)
        for gq in sorted(pending):
            for kind, a1, a2 in pending[gq]:
                emit_transposes(a1, a2)
                tp_done[a1] += 1
                if tp_done[a1] == CP:
                    for t in range(8):
                        for o in range(2):
                            F(2048, lambda tag, ii=a1, tt=t, oo=o:
                              emit_proj(ii, tt, oo, tag))

        # tail: drain remaining fillers at full rate (s banks free now)
        tags = ("s", "s", "y", "y")
        k = 0
        while fillers:
            _, _, fn = fillers.popleft()
            fn(tags[k % 4])
            k += 1

    nc.compile()
    return nc


def _get_nc():
    if "nc" not in _cached:
        _cached["nc"] = _build()
    return _cached["nc"]


def kernel(x, W_qkv, b_qkv, W_proj, b_proj):
    import ml_dtypes
    from concourse.bass_utils import run_bass_kernel_spmd

    bf16 = ml_dtypes.bfloat16
    x = np.asarray(x, dtype=np.float32)
    W_qkv = np.asarray(W_qkv, dtype=np.float32)
    b_qkv = np.asarray(b_qkv, dtype=np.float32)
    W_proj = np.asarray(W_proj, dtype=np.float32)
    b_proj = np.asarray(b_proj, dtype=np.float32)

    in_maps = []
    for core in range(NCORES):
        b, hg = divmod(core, HG)
        qs = slice(DH * hg, DH * (hg + 1))
        ks = slice(D + DH * hg, D + DH * (hg + 1))
        vs = slice(2 * D + DH * hg, 2 * D + DH * (hg + 1))
        def swz_qk(w):   # [D, 512] -> [CP, P, KC, P] (m-major, p-contig)
            return np.ascontiguousarray(
                w.reshape(KC, P, CP, P).transpose(2, 1, 0, 3)).astype(bf16)

        def swz_v(w):    # [D, 512] -> [HH, P, KC, 64]
            return np.ascontiguousarray(
                w.reshape(KC, P, HH, 64).transpose(2, 1, 0, 3)).astype(bf16)

        in_maps.append({
            "xt": np.ascontiguousarray(x[b].T).astype(bf16),
            "wqh": swz_qk(W_qkv[:, qs]),
            "wkh": swz_qk(W_qkv[:, ks]),
            "wvh": swz_v(W_qkv[:, vs]),
            "wph": np.ascontiguousarray(
                W_proj[DH * hg:DH * (hg + 1), :].reshape(CP, P, D)
                .transpose(1, 0, 2)).astype(bf16),
            "bqk": np.concatenate([b_qkv[qs], b_qkv[ks]])[None, :],
        })

    nc = _get_nc()
    res = run_bass_kernel_spmd(nc, in_maps, core_ids=list(range(NCORES)))
    beff = (b_proj.astype(np.float64)
            + b_qkv[2 * D:].astype(np.float64) @ W_proj.astype(np.float64)
            ).astype(np.float32)
    out = np.empty((B, N, D), dtype=np.float32)
    for b in range(B):
        out[b] = (res.results[2 * b]["y"].astype(np.float32)
                  + res.results[2 * b + 1]["y"].astype(np.float32) + beff)
    return out



# revision 9
# speedup vs baseline: 1.1193x; 1.1193x over previous
"""Multi-head attention (B=4, N=2048, D=1024, H=16) on 8 Trainium2 cores — v3.

Sharding: core = (batch b, head-group hg) -> 4 batches x 2 groups of 8 heads.

v3 over v2 (309us -> target ~205us): fp8 DoubleRow matmuls + dual-engine exp.
  - Scores in fp8e4m3 DoubleRow perf mode (0.5 cyc/row vs bf16's 1.0):
    K^T stationary is [64 part, 2 slot, 128] with slot 1 a zeroed plane
    (base partitions are restricted to 0/32/64, so the head's 64 feats sit
    natural in 64 partitions at base 64*(h%2)); the moving Q^T broadcasts
    the same 64 feats into both DR slots (zero stationary annihilates the
    duplicate). 262K -> 131K PE cycles.
  - Q/K projections in fp8 DoubleRow: x and W_q/W_k quantized e4m3 host-side,
    contraction 1024 = 4 DR instructions of [128 part, 2 slot]. The output
    feature->partition permutation is folded into the host weight swizzle so
    the PSUM->SBUF bias-copy writes kt2/qt2 directly (partition-aligned).
    196K -> 33K PE cycles. V projection and PV stay bf16 (error budget).
  - Exp stream split across two engines: ACT does exact Exp on 11/16 tiles;
    DVE computes the other 5/16 via a Schraudolph bit-trick — one
    tensor_scalar (s*A + B) -> int16 whose bit pattern IS bf16 exp(s/8)
    (pattern = 128*(log2e*x + 127) + c, mantissa-linear 2^f approx, ±4% max,
    ~2.3% rms, zero-mean via c; softmax denominator uses the same values so
    the bias cancels). 256 exp tiles: 267us ACT-only -> ~max(183, DVE) us.
  - O normalize/transposes in bf16 (transpose 1.0 cyc/row vs f32's 2.0,
    PSUM->SBUF copies in DVE 2x mode), V copies pair-batched.
PSUM budget (8 banks): s 2x[128,1024]=4, o [128,512]=1, dn [128,512]=1,
y 2x[128,512]=2 (K/Q/V/proj fillers + transposes, rotating).
"""

import sys

if "/opt/trn_rl_repo" not in sys.path:
    sys.path.insert(0, "/opt/trn_rl_repo")

from collections import deque
from contextlib import ExitStack

import numpy as np

B, N, D, H = 4, 2048, 1024, 16
HG = 2                 # head groups (tensor parallel)
NCORES = B * HG        # 8
DH = D // HG           # 512 features per group = 8 heads * 64
HH = H // HG           # 8 heads per core
P = 128
KC = D // P            # 8 contraction chunks over d_model (bf16 V path)
KC2 = 4                # 4 DoubleRow chunks of 256 over d_model (fp8 QK path)
CP = HH // 2           # 4 head pairs per core
TJ = N // P            # 16 key 128-chunks
IB = 1024              # i-block (exp free-dim)
NI = N // IB           # 2
SCALE = (D // H) ** -0.5

# Schraudolph-to-bf16 constants: int16 pattern = s*TRICK_A + TRICK_B, bit
# pattern read as bf16 = exp(s*SCALE)*(1+eps(f)), eps zero-mean, |eps|<4.2%.
LOG2E = 1.4426950408889634
TRICK_A = 128.0 * LOG2E * SCALE
# 128*127 (bias) - 7.3348 (centers ln((1+f)/2^f), mean 0.039720) + 0.5 (round
# via truncation)
TRICK_B = 16256.0 - 7.3348 + 0.5
DVE_EXP_J = frozenset((2, 5, 8, 11, 14))   # 5/16 of exp tiles routed to DVE

_cached = {}


def _build():
    import concourse.mybir as mybir
    import concourse.tile as tile
    from concourse import bacc, masks

    f32 = mybir.dt.float32
    bf16 = mybir.dt.bfloat16
    fp8 = mybir.dt.float8e4
    i16 = mybir.dt.int16
    AF = mybir.ActivationFunctionType
    DR = mybir.MatmulPerfMode.DoubleRow
    MUL = mybir.AluOpType.mult
    ADD = mybir.AluOpType.add

    nc = bacc.Bacc("TRN2", target_bir_lowering=False, debug=False,
                   enable_asserts=False)

    # host-packed, partition-major DRAM inputs (all DMAs are contiguous runs)
    xt = nc.dram_tensor("xt", (P, KC, N), bf16, kind="ExternalInput").ap()
    xt8 = nc.dram_tensor("xt8", (P, 2, KC2, N), fp8, kind="ExternalInput").ap()
    wq8 = nc.dram_tensor("wq8", (P, 4, KC2, 2, P), fp8,
                         kind="ExternalInput").ap()
    wk8 = nc.dram_tensor("wk8", (P, 4, KC2, 2, P), fp8,
                         kind="ExternalInput").ap()
    wvh = nc.dram_tensor("wvh", (P, HH, KC, 64), bf16,
                         kind="ExternalInput").ap()
    wph = nc.dram_tensor("wph", (P, CP, D), bf16, kind="ExternalInput").ap()
    bqh = nc.dram_tensor("bqh", (P, 8), f32, kind="ExternalInput").ap()
    y = nc.dram_tensor("y", (N, D), bf16, kind="ExternalOutput").ap()

    with tile.TileContext(nc) as tc, ExitStack() as ctx:
        const = ctx.enter_context(tc.tile_pool(name="const", bufs=1))
        persist = ctx.enter_context(tc.tile_pool(name="persist", bufs=1))
        ppool = ctx.enter_context(tc.tile_pool(name="pp", bufs=8))
        ospool = ctx.enter_context(tc.tile_pool(name="osb", bufs=2))
        otpool = ctx.enter_context(tc.tile_pool(name="ot", bufs=2))
        dpool = ctx.enter_context(tc.tile_pool(name="dv", bufs=2))
        ypool = ctx.enter_context(tc.tile_pool(name="yb", bufs=5))
        psp = ctx.enter_context(tc.tile_pool(name="psp", bufs=1, space="PSUM"))

        # ---- consts ----
        wconst = const.tile([P, 512], bf16)
        nc.vector.memset(wconst[:], 0.0)
        ones_bf = const.tile([P, 16], bf16)
        nc.vector.memset(ones_bf[:], 1.0)

        # ---- persistent SBUF ----
        xt_sb = persist.tile([P, KC, N], bf16)          # bf16 x^T (V path)
        xt8_sb = persist.tile([P, 2, KC2, N], fp8)      # fp8 x^T, DR pairs
        wq8_sb = persist.tile([P, 4, KC2, 2, P], fp8)
        wk8_sb = persist.tile([P, 4, KC2, 2, P], fp8)
        wv_sb = persist.tile([P, HH, KC, 64], bf16)
        wp_sb = persist.tile([P, CP, D], bf16)
        # K^T fp8 [64*b+f64, m, slot, tok]: chunk m = 2*quad + h4//2 holds
        # heads h4 = 2*(m%2) + b at partition blocks b*64; slot 1 is zeros
        # (the DR stationary's second k-tile). Q^T has no slot dim — the
        # moving operand broadcasts slot 0 into both DR slots.
        kt2 = persist.tile([P, 4, 2, N], fp8)
        qt2 = persist.tile([P, 4, N], fp8)
        vsb = persist.tile([P, TJ, HH, 64], bf16)  # V [keytok128, j, h, feat]
        bqk_sb = const.tile([P, 1, 8], f32)        # [q, 1, (t=q/k)*4+m]

        # ---- DMAs (prefix-critical first) ----
        nc.sync.dma_start(wk8_sb[:], wk8)
        nc.sync.dma_start(xt8_sb[:, :, :, 0:1024], xt8[:, :, :, 0:1024])
        nc.sync.dma_start(wq8_sb[:], wq8)
        nc.sync.dma_start(bqk_sb[:, 0, :], bqh)
        nc.sync.dma_start(xt_sb[:, :, 0:256], xt[:, :, 0:256])
        nc.sync.dma_start(wv_sb[:, 0:2], wvh[:, 0:2])
        nc.sync.dma_start(xt8_sb[:, :, :, 1024:2048], xt8[:, :, :, 1024:2048])
        nc.sync.dma_start(wv_sb[:, 2:HH], wvh[:, 2:HH])
        nc.sync.dma_start(xt_sb[:, :, 256:768], xt[:, :, 256:768])
        nc.sync.dma_start(xt_sb[:, :, 768:1408], xt[:, :, 768:1408])
        nc.sync.dma_start(xt_sb[:, :, 1408:2048], xt[:, :, 1408:2048])
        nc.sync.dma_start(wp_sb[:], wph)

        # zero the DR slot-1 planes of kt2 on the (otherwise idle) gpsimd
        # engine before the first scores touch them; quad0 first.
        nc.gpsimd.memset(kt2[:, 0:2, 1, :], 0.0)
        nc.gpsimd.memset(kt2[:, 2:4, 1, :], 0.0)

        # identity (bf16: 1.0 cyc/row transposes) for PE transposes
        ident_bf = const.tile([P, P], bf16)
        masks.make_identity(nc, ident_bf[:])

        # preload the exp table while ACT is idle
        dummy = const.tile([1, 16], f32)
        nc.scalar.activation(dummy[:], ones_bf[0:1, :], AF.Exp)

        # persistent PSUM: denominators. Zeroed once; every denom matmul
        # accumulates (see v2 docstring for the zero-region constraint).
        dn = psp.tile([P, 512], f32, tag="d", bufs=1, name="dn")
        nc.tensor.matmul(dn[:, 0:NI * HH * 8], wconst[:, 0:P],
                         wconst[:, 0:NI * HH * 8], start=True, stop=False,
                         skip_group_check=True)

        # warmup: absorb the PE p-state ramp under the initial DMA wait
        for _ in range(12):
            wm = psp.tile([P, IB], f32, tag="s", bufs=2, name="wm")
            nc.tensor.matmul(wm[:, 0:512], wconst[:, 0:P], wconst[:],
                             start=True, stop=True, skip_group_check=True)

        # ================= emission helpers =================
        def _bufs(tag):
            return 2 if tag in ("s", "y") else 1

        def emit_k_chunk(m, n, tag):
            """kt2[:, quad, ab, n*512:...] — 4 DR matmuls + bias-add copy."""
            c0 = n * 512
            pt = psp.tile([P, 512], f32, tag=tag, bufs=_bufs(tag), name="ptk")
            for kk in range(KC2):
                nc.tensor.matmul(pt[:], wk8_sb[:, m, kk],
                                 xt8_sb[:, :, kk, c0:c0 + 512],
                                 start=(kk == 0), stop=(kk == KC2 - 1),
                                 perf_mode=DR)
            nc.vector.tensor_scalar_add(
                kt2[:, m, 0, c0:c0 + 512], pt[:],
                bqk_sb[:, 0, 4 + m:5 + m])

        def emit_q_chunk(m, n, tag):
            c0 = n * 512
            pt = psp.tile([P, 512], f32, tag=tag, bufs=_bufs(tag), name="ptq")
            for kk in range(KC2):
                nc.tensor.matmul(pt[:], wq8_sb[:, m, kk],
                                 xt8_sb[:, :, kk, c0:c0 + 512],
                                 start=(kk == 0), stop=(kk == KC2 - 1),
                                 perf_mode=DR)
            nc.vector.tensor_scalar_add(
                qt2[:, m, c0:c0 + 512], pt[:],
                bqk_sb[:, 0, m:m + 1])

        def emit_v_slice(c, g, tag):
            """vsb[:, g, 2c:2c+2, :] — V for head pair c, keytok chunk g.
            Both heads accumulate in one bank (first matmul's start=True
            zeroes the whole bank; the rest accumulate, skip_group_check)."""
            pv = psp.tile([P, 512], f32, tag=tag, bufs=_bufs(tag),
                          name="ptv")
            for hi in range(2):
                hh = 2 * c + hi
                for k in range(KC):
                    first = hi == 0 and k == 0
                    nc.tensor.matmul(
                        pv[:, hi * 64:hi * 64 + 64],
                        xt_sb[:, k, g * P:(g + 1) * P],
                        wv_sb[:, hh, k, :],
                        start=first, stop=(hi == 1 and k == KC - 1),
                        skip_group_check=not first)
            nc.vector.tensor_copy(
                vsb[:, g, 2 * c:2 * c + 2, :].rearrange("p a b -> p (a b)"),
                pv[:, 0:128])

        ot_tiles = {}

        def emit_proj(i, t, o, tag):
            ot_i = ot_tiles[i]
            yp = psp.tile([P, 512], f32, tag=tag, bufs=_bufs(tag), name="yp")
            for cc in range(CP):
                nc.tensor.matmul(yp[:], ot_i[:, cc, t * P:(t + 1) * P],
                                 wp_sb[:, cc, o * 512:(o + 1) * 512],
                                 start=(cc == 0), stop=(cc == CP - 1))
            ysb = ypool.tile([P, 512], bf16, tag="ysb", name="ysb")
            nc.vector.tensor_copy(ysb[:], yp[:])
            r0 = i * IB + t * P
            nc.sync.dma_start(y[r0:r0 + P, o * 512:(o + 1) * 512], ysb[:])

        # filler queue: (deadline_step, pe_cycles, fn(tag)) in deadline order
        fillers = deque()

        def F(cycles, fn, deadline=10**9):
            fillers.append((deadline, cycles, fn))

        def pump(credit, step=-1):
            while fillers and (fillers[0][0] <= step
                               or fillers[0][1] <= credit):
                _, cyc, fn = fillers.popleft()
                fn("y")
                credit -= cyc
            return credit

        # ================= prefix =================
        emit_k_chunk(0, 0, "s")
        emit_k_chunk(1, 0, "s")
        emit_q_chunk(0, 0, "s")
        emit_q_chunk(1, 0, "s")
        emit_q_chunk(0, 1, "s")
        emit_q_chunk(1, 1, "s")
        emit_v_slice(0, 0, "s")
        emit_v_slice(0, 1, "s")

        # unit order (i, hh): i1 units of a pair consume no new K/V fillers
        NU = NI * HH                      # 16 units
        seq = [(0, 0), (0, 1), (1, 0), (1, 1),
               (0, 2), (0, 3), (1, 2), (1, 3),
               (0, 4), (0, 5), (0, 6), (0, 7),
               (1, 4), (1, 5), (1, 6), (1, 7)]
        pos_of = {u: p for p, u in enumerate(seq)}

        # ---- filler events (deadline-sorted) ----
        events = []   # (deadline_step, pe_cycles, fn)
        for c in range(CP):
            p0 = 16 * pos_of[(0, 2 * c)]
            for g in range(TJ):
                if c == 0 and g < 2:
                    continue   # prefix
                events.append((p0 + g, 1024,
                               lambda tag, c=c, g=g: emit_v_slice(c, g, tag)))
        for m in range(4):
            quad = m // 2
            pk = 16 * pos_of[(0, 4 * quad)]
            for n in range(4):
                if m < 2 and n == 0:
                    continue   # prefix
                events.append((max(0, pk + 4 * n - 3 + (m % 2)), 1024,
                               lambda tag, m=m, n=n: emit_k_chunk(m, n, tag)))
            for n in range(4):
                if m < 2 and n < 2:
                    continue   # prefix
                blk = n // 2
                dl = 16 * pos_of[(blk, 4 * quad)] - 3 + (m % 2)
                events.append((max(0, dl), 1024,
                               lambda tag, m=m, n=n: emit_q_chunk(m, n, tag)))
        events.sort(key=lambda e: e[0])
        for dl, cyc, fn in events:
            F(cyc, fn, deadline=dl)

        # ================= attention units =================
        p_of = {}      # (u, j) -> p tile
        o_ps_of = {}   # u -> O psum accumulator
        osb_of = {}    # (i, c) -> normalized-O sbuf tile

        def emit_scores_exp(u, j):
            i, hh = seq[u]
            quad, h4 = divmod(hh, 4)
            m = 2 * quad + h4 // 2
            r0 = 64 * (h4 % 2)
            s = psp.tile([P, IB], f32, tag="s", bufs=2, name="s")
            for iq in range(2):
                c0 = i * IB + iq * 512
                nc.tensor.matmul(
                    s[:, iq * 512:(iq + 1) * 512],
                    kt2[r0:r0 + 64, m, :, j * P:(j + 1) * P],
                    qt2[r0:r0 + 64, m, c0:c0 + 512][:, None, :]
                    .broadcast_to([64, 2, 512]),
                    start=True, stop=True, perf_mode=DR)
            p = ppool.tile([P, IB], bf16, tag="p", name="p")
            if j in DVE_EXP_J:
                nc.vector.tensor_scalar(p[:].bitcast(i16), s[:],
                                        TRICK_A, TRICK_B, op0=MUL, op1=ADD)
            else:
                nc.scalar.activation(p[:], s[:], AF.Exp, scale=SCALE)
            p_of[(u, j)] = p

        def emit_pv(u, j):
            i, hh = seq[u]
            if j == 0:
                o_ps_of[u] = psp.tile([P, 512], f32, tag="o", bufs=1,
                                      name="ops")
                nc.tensor.matmul(o_ps_of[u][:], wconst[:, 0:P], wconst[:],
                                 start=True, stop=False,
                                 skip_group_check=True)
            o_ps = o_ps_of[u]
            p = p_of.pop((u, j))
            for t in range(8):
                nc.tensor.matmul(o_ps[:, t * 64:(t + 1) * 64],
                                 p[:, t * P:(t + 1) * P], vsb[:, j, hh, :],
                                 start=False, stop=False,
                                 skip_group_check=True)
            for t in range(8):
                nc.tensor.matmul(dn[:, u * 8 + t:u * 8 + t + 1],
                                 p[:, t * P:(t + 1) * P], ones_bf[:, 0:1],
                                 start=False, stop=False,
                                 skip_group_check=True)

        def emit_norm(u):
            i, hh = seq[u]
            c, hp = divmod(hh, 2)
            o_ps = o_ps_of.pop(u)
            rcp = dpool.tile([P, 8], f32, tag="rcp", name="rcp")
            nc.vector.reciprocal(rcp[:], dn[:, u * 8:u * 8 + 8])
            if hp == 0:
                osb_of[(i, c)] = ospool.tile([P, 8, 2, 64], bf16, tag="osb",
                                             name="osb")
            osb = osb_of[(i, c)]
            nc.vector.tensor_mul(
                osb[:, :, hp, :],
                o_ps[:].rearrange("p (t f) -> p t f", f=64),
                rcp[:, :, None].broadcast_to([P, 8, 64]))

        def emit_transposes(i, c):
            """O pair-block [qtok, 128feat] -> ot [128feat, qtok], bf16 PE
            transposes into a bf16 view of the f32-zeroed psum bank."""
            osb = osb_of.pop((i, c))
            for g in range(2):
                yslot = psp.tile([P, 512], f32, tag="y", bufs=2, name="tp")
                nc.tensor.matmul(yslot[:], wconst[:, 0:P], wconst[:],
                                 start=True, stop=False,
                                 skip_group_check=True)
                for tt in range(4):
                    t = g * 4 + tt
                    nc.tensor.matmul(
                        yslot[:, tt * 64:(tt + 1) * 64].bitcast(bf16),
                        osb[:, t, :, :].rearrange("p a b -> p (a b)"),
                        ident_bf[:], is_transpose=True,
                        start=False, stop=False, skip_group_check=True)
                nc.vector.tensor_copy(
                    ot_tiles[i][:, c, g * 512:(g + 1) * 512],
                    yslot[:, 0:256].bitcast(bf16))

        # PV trails the scores/exp stream by PVLAG steps; transposes trail
        # the norm so the norm chain never head-of-line blocks scores.
        CREDIT_PER_STEP = 620
        CREDIT_CAP = 3000
        PVLAG = 5
        credit = -2200    # delay the first credit pops past the prefix chain
        pending = {}
        tp_done = {0: 0, 1: 0}
        for g in range(NU * TJ + PVLAG + 4):
            credit = pump(credit, g)   # deadline-forced pops
            if g < NU * TJ:
                u, j = divmod(g, TJ)
                if j == 0:
                    i, hh = seq[u]
                    if hh == 0 and i not in ot_tiles:
                        ot_tiles[i] = otpool.tile([P, CP, IB], bf16,
                                                  tag="ot", name="ot")
                emit_scores_exp(u, j)
            gp = g - PVLAG
            if 0 <= gp < NU * TJ:
                up, jp = divmod(gp, TJ)
                if jp == 0:
                    pass                      # deferred: paired with j1
                elif jp == 1:
                    emit_pv(up, 0)
                    emit_pv(up, 1)
                else:
                    emit_pv(up, jp)
                if jp == TJ - 1:
                    iup, hup = seq[up]
                    emit_norm(up)
                    if hup % 2 == 1:
                        pending.setdefault(g + 6, []).append(
                            ("tp", iup, hup // 2))
            for kind, a1, a2 in pending.pop(g, []):
                emit_transposes(a1, a2)
                tp_done[a1] += 1
                if tp_done[a1] == CP:
                    for t in range(8):
                        for o in range(2):
                            F(2048, lambda tag, ii=a1, tt=t, oo=o:
                              emit_proj(ii, tt, oo, tag))
            credit = min(credit + CREDIT_PER_STEP, CREDIT_CAP)
            credit = pump(credit, g)
        for gq in sorted(pending):
            for kind, a1, a2 in pending[gq]:
                emit_transposes(a1, a2)
                tp_done[a1] += 1
                if tp_done[a1] == CP:
                    for t in range(8):
                        for o in range(2):
                            F(2048, lambda tag, ii=a1, tt=t, oo=o:
                              emit_proj(ii, tt, oo, tag))

        # tail: drain remaining fillers at full rate (s banks free now)
        tags = ("s", "s", "y", "y")
        k = 0
        while fillers:
            _, _, fn = fillers.popleft()
            fn(tags[k % 4])
            k += 1

    nc.compile()
    return nc


def _get_nc():
    if "nc" not in _cached:
        _cached["nc"] = _build()
    return _cached["nc"]


def kernel(x, W_qkv, b_qkv, W_proj, b_proj):
    import ml_dtypes
    from concourse.bass_utils import run_bass_kernel_spmd

    bf16 = ml_dtypes.bfloat16
    fp8 = ml_dtypes.float8_e4m3
    x = np.asarray(x, dtype=np.float32)
    W_qkv = np.asarray(W_qkv, dtype=np.float32)
    b_qkv = np.asarray(b_qkv, dtype=np.float32)
    W_proj = np.asarray(W_proj, dtype=np.float32)
    b_proj = np.asarray(b_proj, dtype=np.float32)

    # feat permutation: output partition q of stationary chunk m holds
    # feature f = 256*(m//2) + 64*(2*(m%2) + q//64) + q%64
    mm, qq = np.meshgrid(np.arange(4), np.arange(P), indexing="ij")
    feat_idx = 256 * (mm // 2) + 64 * (2 * (mm % 2) + qq // 64) + qq % 64

    def pack_qk8(w):     # [D, 512] -> [P, 4m, KC2, 2i, 128q] fp8
        a = w.reshape(KC2, 2, P, DH).transpose(2, 0, 1, 3)  # [p, kk, i, f]
        a = a[..., feat_idx]                                # [p,kk,i,m,q]
        return np.ascontiguousarray(
            a.transpose(0, 3, 1, 2, 4)).astype(fp8)

    in_maps = []
    for core in range(NCORES):
        b, hg = divmod(core, HG)
        qs = slice(DH * hg, DH * (hg + 1))
        ks = slice(D + DH * hg, D + DH * (hg + 1))
        vs = slice(2 * D + DH * hg, 2 * D + DH * (hg + 1))
        xT = np.ascontiguousarray(x[b].T)                   # [D, N]
        bq = np.concatenate([
            b_qkv[qs][feat_idx].T,                          # [128, 4] q
            b_qkv[ks][feat_idx].T,                          # [128, 4] k
        ], axis=1).astype(np.float32)                       # [128, 8]

        in_maps.append({
            "xt": np.ascontiguousarray(
                xT.reshape(KC, P, N).transpose(1, 0, 2)).astype(bf16),
            "xt8": np.ascontiguousarray(
                xT.reshape(KC2, 2, P, N).transpose(2, 1, 0, 3)).astype(fp8),
            "wq8": pack_qk8(W_qkv[:, qs]),
            "wk8": pack_qk8(W_qkv[:, ks]),
            "wvh": np.ascontiguousarray(
                W_qkv[:, vs].reshape(KC, P, HH, 64)
                .transpose(1, 2, 0, 3)).astype(bf16),
            "wph": np.ascontiguousarray(
                W_proj[DH * hg:DH * (hg + 1), :].reshape(CP, P, D)
                .transpose(1, 0, 2)).astype(bf16),
            "bqh": np.ascontiguousarray(bq),
        })

    nc = _get_nc()
    res = run_bass_kernel_spmd(nc, in_maps, core_ids=list(range(NCORES)))
    beff = (b_proj.astype(np.float64)
            + b_qkv[2 * D:].astype(np.float64) @ W_proj.astype(np.float64)
            ).astype(np.float32)
    out = np.empty((B, N, D), dtype=np.float32)
    for b in range(B):
        out[b] = (res.results[2 * b]["y"].astype(np.float32)
                  + res.results[2 * b + 1]["y"].astype(np.float32) + beff)
    return out
